# revision 16
# baseline (speedup 1.0000x reference)
"""Trainium2 Bass kernel for nn_AttentionBlock (sparse_attention, no-softmax).

Computation (per batch b):
    qh = (q @ Wq^T) split into 16 heads of dk=64     [S, D] -> [H, S, DK]
    kh, vh likewise
    scores = (qh @ kh^T) / sqrt(DK)                  [H, S, S]
    p      = scores * A^T                            (elementwise structural mask)
    x      = p @ vh                                  [H, S, DK] -> [S, D]
    out    = x @ Wo^T + bo                           [S, D]

Sharding over 8 NeuronCores: data-parallel over batch (B=2) x tensor-parallel
over heads (16 heads -> 4 per core). Each core projects q/k/v for its 4 heads
(column-parallel), runs masked attention for them, and applies its 256-column
slice of the output projection (row-parallel), producing a full-shape partial
output. Host sums the 4 partials per batch.

Layout strategy: activations are shipped pre-transposed ([D, S]) so every
matmul contraction dim lands on SBUF partitions with no on-device transposes.
Matmuls run in float32r (TF32-like PE mode, ~1e-4 relative error, 4x faster
than strict fp32). The mask multiply folds the 1/sqrt(DK) scale into A on the
host; it is split 3:1 between the Vector engine (straight out of PSUM) and
GPSIMD (via a ScalarE PSUM->SBUF bounce) because it is the throughput-critical
elementwise stage. Projection work for the next/previous query block is
interleaved into the attention loop so the PE never sits behind the DVE.
"""

import numpy as np

import concourse.mybir as mybir
import concourse.tile as tile
from concourse import bacc, bass_utils

B, S, D, H = 2, 2048, 1024, 16
NCORES = 8
GROUPS = NCORES // B          # 4 head-groups
HPC = H // GROUPS             # 4 heads per core
DK = D // H                   # 64
HD = HPC * DK                 # 256 head-dim columns per core
SCALE = 1.0 / np.sqrt(DK)

P = 128                       # SBUF partitions
QB = 512                      # query block
NQB = S // QB                 # 4
KBLK = 128                    # key block
NKB = S // KBLK               # 16
NKT = D // P                  # 8 contraction chunks for projections
AGRP = 4                      # key-blocks per A-tile DMA / interleave group
NGRP = NKB // AGRP            # 4 groups

GP_OFFLOAD = True             # route head 3's mask multiply via ScalarE+GPSIMD

f32 = mybir.dt.float32
f32r = mybir.dt.float32r

_CACHED = None  # built module, reused across kernel() calls
TRACE = False         # set True (e.g. from test.py) to profile the NEFF
LAST_RESULTS = None   # BassKernelResults of the most recent run


def _build():
    nc = bacc.Bacc("TRN2", target_bir_lowering=False)

    qT = nc.dram_tensor("qT", [D, S], f32r, kind="ExternalInput")
    kT = nc.dram_tensor("kT", [D, S], f32r, kind="ExternalInput")
    vT = nc.dram_tensor("vT", [D, S], f32r, kind="ExternalInput")
    Asc = nc.dram_tensor("Asc", [S, S], f32, kind="ExternalInput")
    wq = nc.dram_tensor("wq", [D, HD], f32r, kind="ExternalInput")
    wk = nc.dram_tensor("wk", [D, HD], f32r, kind="ExternalInput")
    wv = nc.dram_tensor("wv", [D, HD], f32r, kind="ExternalInput")
    wo = nc.dram_tensor("wo", [HD, D], f32r, kind="ExternalInput")
    out = nc.dram_tensor("out", [S, D], f32, kind="ExternalOutput")

    qT_r = qT.rearrange("(kt p) s -> p kt s", p=P)
    kT_r = kT.rearrange("(kt p) s -> p kt s", p=P)
    vT_r = vT.rearrange("(kt p) s -> p kt s", p=P)
    wq_r = wq.rearrange("(kt p) c -> p kt c", p=P)
    wk_r = wk.rearrange("(kt p) c -> p kt c", p=P)
    wv_r = wv.rearrange("(kt p) c -> p kt c", p=P)
    wo_r = wo.rearrange("(ck p) e -> p ck e", p=P)
    A_r = Asc.rearrange("(kb p) q -> p kb q", p=P)

    with tile.TileContext(nc) as tc:
        with (
            tc.tile_pool(name="persist", bufs=1) as pp,
            tc.tile_pool(name="stream", bufs=2) as sp,
            tc.tile_pool(name="psU", bufs=4, space="PSUM") as psU,   # univ [128,512]
            tc.tile_pool(name="psX", bufs=1, space="PSUM") as psX,   # xT accumulators
        ):
            # ---- resident weights (k first: k-proj starts the pipeline) ----------
            wk_sb = pp.tile([P, NKT, HD], f32r, tag="wk")
            wv_sb = pp.tile([P, NKT, HD], f32r, tag="wv")
            wq_sb = pp.tile([P, NKT, HD], f32r, tag="wq")
            wo_sb = pp.tile([P, HD // P, D], f32r, tag="wo")

            khT_sb = pp.tile([DK, HPC, S], f32r, tag="khT")   # [dk, h, ks]
            vh_sb = pp.tile([P, NKB, HD], f32r, tag="vh")     # [ks%128, kb, c]

            def kv_proj(st, kT_pre=None, vT_pre=None):
                """Project k and v for s-slice st (512 rows of S)."""
                sl = slice(st * QB, (st + 1) * QB)
                kT_sb = kT_pre
                if kT_sb is None:
                    kT_sb = sp.tile([P, NKT, QB], f32r, tag="xin", bufs=3, name="kT_sb")
                    nc.sync.dma_start(kT_sb[:], kT_r[:, :, sl])
                for ct in range(HD // P):
                    pk = psU.tile([P, QB], f32, tag="u", name="pk")
                    for kt in range(NKT):
                        nc.tensor.matmul(
                            pk[:], wk_sb[:, kt, ct * P:(ct + 1) * P], kT_sb[:, kt, :],
                            start=(kt == 0), stop=(kt == NKT - 1),
                        )
                    nc.scalar.copy(khT_sb[:, 2 * ct, sl], pk[0:DK, :])
                    nc.vector.tensor_copy(khT_sb[:, 2 * ct + 1, sl], pk[DK:P, :])

                vT_sb = vT_pre
                if vT_sb is None:
                    vT_sb = sp.tile([P, NKT, QB], f32r, tag="xin", bufs=3, name="vT_sb")
                    nc.sync.dma_start(vT_sb[:], vT_r[:, :, sl])
                for ssub in range(QB // P):
                    kb = st * (QB // P) + ssub
                    pv = psU.tile([P, HD], f32, tag="u", name="pv")
                    for kt in range(NKT):
                        nc.tensor.matmul(
                            pv[:], vT_sb[:, kt, ssub * P:(ssub + 1) * P], wv_sb[:, kt, :],
                            start=(kt == 0), stop=(kt == NKT - 1),
                        )
                    nc.scalar.copy(vh_sb[:, kb, :], pv[:])

            def q_proj(qb):
                """Project q for query block qb -> qhT tile [dk, h, 512]."""
                qsl = slice(qb * QB, (qb + 1) * QB)
                qT_sb = sp.tile([P, NKT, QB], f32r, tag="xin", bufs=3, name="qT_sb")
                nc.sync.dma_start(qT_sb[:], qT_r[:, :, qsl])
                qhT_sb = sp.tile([DK, HPC, QB], f32r, tag="qh", name="qhT_sb")
                for ct in range(HD // P):
                    pq = psU.tile([P, QB], f32, tag="u", name="pq")
                    for kt in range(NKT):
                        nc.tensor.matmul(
                            pq[:], wq_sb[:, kt, ct * P:(ct + 1) * P], qT_sb[:, kt, :],
                            start=(kt == 0), stop=(kt == NKT - 1),
                        )
                    nc.scalar.copy(qhT_sb[:, 2 * ct, :], pq[0:DK, :])
                    nc.vector.tensor_copy(qhT_sb[:, 2 * ct + 1, :], pq[DK:P, :])
                return qhT_sb

            def o_proj_chain(xts, qb, ssub):
                """One 128-row slice of the output projection for query block qb."""
                osb = sp.tile([P, D], f32, tag="osb", bufs=3, name="osb")
                for et in range(D // QB):
                    po = psU.tile([P, QB], f32, tag="u", name="po")
                    for ck in range(HD // P):
                        nc.tensor.matmul(
                            po[:],
                            xts[:, ck, ssub * P:(ssub + 1) * P],
                            wo_sb[:, ck, et * QB:(et + 1) * QB],
                            start=(ck == 0), stop=(ck == HD // P - 1),
                        )
                    nc.scalar.copy(osb[:, et * QB:(et + 1) * QB], po[:])
                nc.sync.dma_start(
                    out[qb * QB + ssub * P:qb * QB + (ssub + 1) * P, :], osb[:]
                )

            # ---- pipeline ---------------------------------------------------------
            # prologue DMAs in dependency-first order so the PE starts ASAP
            nc.sync.dma_start(wk_sb[:], wk_r[:])
            kT0 = sp.tile([P, NKT, QB], f32r, tag="xin", bufs=3, name="kT_sb")
            nc.sync.dma_start(kT0[:, 0:NKT // 2, :], kT_r[:, 0:NKT // 2, 0:QB])
            nc.sync.dma_start(kT0[:, NKT // 2:, :], kT_r[:, NKT // 2:, 0:QB])
            nc.sync.dma_start(wv_sb[:], wv_r[:])
            vT0 = sp.tile([P, NKT, QB], f32r, tag="xin", bufs=3, name="vT_sb")
            nc.sync.dma_start(vT0[:], vT_r[:, :, 0:QB])
            nc.sync.dma_start(wq_sb[:], wq_r[:])
            A0 = sp.tile([P, AGRP, QB], f32, tag="A", bufs=2, name="A_sb")
            nc.sync.dma_start(A0[:], A_r[:, 0:AGRP, 0:QB])

            kv_proj(0, kT_pre=kT0, vT_pre=vT0)
            qhT_cur = q_proj(0)
            nc.sync.dma_start(wo_sb[:], wo_r[:])
            pend_xts = None    # (xts tile, qb) awaiting output projection
            qhT_next = None

            for qb in range(NQB):
                qsl = slice(qb * QB, (qb + 1) * QB)
                xt = psX.tile([DK, HPC, QB], f32, tag="xt", name="xt")  # 4 banks
                xts = sp.tile([P, HD // P, QB], f32r, tag="xts", bufs=2, name="xts")
                for kbg in range(NGRP):
                    if qb == 0 and kbg == 0:
                        A_sb = A0
                    else:
                        A_sb = sp.tile([P, AGRP, QB], f32, tag="A", bufs=2, name="A_sb")
                        nc.sync.dma_start(
                            A_sb[:], A_r[:, kbg * AGRP:(kbg + 1) * AGRP, qsl]
                        )
                    for i in range(AGRP):
                        kb = kbg * AGRP + i
                        pts = []
                        for h in range(HPC):
                            sc = psU.tile([P, QB], f32, tag="u", name="sc")
                            nc.tensor.matmul(
                                sc[:],
                                khT_sb[:, h, kb * KBLK:(kb + 1) * KBLK],
                                qhT_cur[:, h, :],
                                start=True, stop=True,
                            )
                            pt = sp.tile([P, QB], f32r, tag="pt", bufs=6, name="pt")
                            if GP_OFFLOAD and h >= HPC - 2:
                                sc_sb = sp.tile([P, QB], f32, tag="scb", bufs=3,
                                                name="sc_sb")
                                nc.scalar.copy(sc_sb[:], sc[:])
                                nc.gpsimd.tensor_tensor(
                                    pt[:], sc_sb[:], A_sb[:, i, :],
                                    mybir.AluOpType.mult,
                                )
                            else:
                                nc.vector.tensor_tensor(
                                    pt[:], sc[:], A_sb[:, i, :], mybir.AluOpType.mult
                                )
                            pts.append(pt)
                        for h in range(HPC):
                            nc.tensor.matmul(
                                xt[:, h, :],
                                vh_sb[:, kb, h * DK:(h + 1) * DK],
                                pts[h],
                                start=(kb == 0), stop=(kb == NKB - 1),
                            )
                            if kb == NKB - 1:
                                # drain this head's accumulator immediately so
                                # the next q-block's first matmul isn't gated
                                # on a serial 4-copy epilogue
                                dst = xts[(h % 2) * DK:(h % 2 + 1) * DK, h // 2, :]
                                if h % 2 == 0:
                                    nc.scalar.copy(dst, xt[:, h, :])
                                else:
                                    nc.vector.tensor_copy(dst, xt[:, h, :])

                    # interleave independent work into the DVE-bound loop
                    if qb == 0:
                        if kbg < NGRP - 1:
                            kv_proj(kbg + 1)
                        else:
                            qhT_next = q_proj(1)
                    else:
                        if kbg < 2 and pend_xts is not None:
                            xts_p, qb_p = pend_xts
                            o_proj_chain(xts_p, qb_p, 2 * kbg)
                            o_proj_chain(xts_p, qb_p, 2 * kbg + 1)
                            if kbg == 1:
                                pend_xts = None
                        elif kbg == NGRP - 1 and qb < NQB - 1:
                            qhT_next = q_proj(qb + 1)

                pend_xts = (xts, qb)
                qhT_cur, qhT_next = qhT_next, None

            # drain the last query block's output projection
            xts_p, qb_p = pend_xts
            for ssub in range(QB // P):
                o_proj_chain(xts_p, qb_p, ssub)

    nc.compile()
    return nc


def _numpy_fallback(q, k, v, A, Wq, bq, Wk, bk, Wv, bv, Wo, bo):
    def proj(x, W, b):
        y = x @ W.T + b
        return y.reshape(B, S, H, DK).transpose(0, 2, 1, 3)

    qh, kh, vh = proj(q, Wq, bq), proj(k, Wk, bk), proj(v, Wv, bv)
    scores = np.einsum("bhqd,bhkd->bhqk", qh, kh) * np.float32(SCALE)
    p = scores * A.T
    x = np.einsum("bhqk,bhkd->bhqd", p, vh)
    x = x.transpose(0, 2, 1, 3).reshape(B, S, D)
    return (x @ Wo.T + bo).astype(np.float32)


def kernel(**inputs):
    q = np.asarray(inputs["q"], dtype=np.float32)
    k = np.asarray(inputs["k"], dtype=np.float32)
    v = np.asarray(inputs["v"], dtype=np.float32)
    A = np.asarray(inputs["A"], dtype=np.float32)
    Wq = np.asarray(inputs["Wq"], dtype=np.float32)
    Wk = np.asarray(inputs["Wk"], dtype=np.float32)
    Wv = np.asarray(inputs["Wv"], dtype=np.float32)
    Wo = np.asarray(inputs["Wo"], dtype=np.float32)
    bq, bk, bv, bo = (np.asarray(inputs[n], dtype=np.float32) for n in ("bq", "bk", "bv", "bo"))

    # The device kernel folds zero biases away (spec fills them with zeros);
    # fall back to a host reference in the (unused) nonzero-bias case.
    if any(np.any(b) for b in (bq, bk, bv)):
        return _numpy_fallback(q, k, v, A, Wq, bq, Wk, bk, Wv, bv, Wo, bo)

    global _CACHED
    if _CACHED is None:
        _CACHED = _build()
    nc = _CACHED

    Asc = np.ascontiguousarray(A * np.float32(SCALE))
    in_maps = []
    for c in range(NCORES):
        b, g = divmod(c, GROUPS)
        hsl = slice(g * HD, (g + 1) * HD)
        in_maps.append({
            "qT": np.ascontiguousarray(q[b].T),
            "kT": np.ascontiguousarray(k[b].T),
            "vT": np.ascontiguousarray(v[b].T),
            "Asc": Asc,
            "wq": np.ascontiguousarray(Wq[hsl].T),
            "wk": np.ascontiguousarray(Wk[hsl].T),
            "wv": np.ascontiguousarray(Wv[hsl].T),
            "wo": np.ascontiguousarray(Wo[:, hsl].T),
        })

    res = bass_utils.run_bass_kernel_spmd(
        nc, in_maps, core_ids=list(range(NCORES)), trace=TRACE
    )
    global LAST_RESULTS
    LAST_RESULTS = res

    out = np.zeros((B, S, D), dtype=np.float32)
    for c in range(NCORES):
        out[c // GROUPS] += res.results[c]["out"]
    out += bo
    return out


if __name__ == "__main__":
    rng = np.random.default_rng(0)
    ins = {
        "q": rng.standard_normal((B, S, D), dtype=np.float32),
        "k": rng.standard_normal((B, S, D), dtype=np.float32),
        "v": rng.standard_normal((B, S, D), dtype=np.float32),
        "A": rng.random((S, S), dtype=np.float32),
        "Wq": rng.standard_normal((D, D), dtype=np.float32) / 32,
        "bq": np.zeros(D, np.float32),
        "Wk": rng.standard_normal((D, D), dtype=np.float32) / 32,
        "bk": np.zeros(D, np.float32),
        "Wv": rng.standard_normal((D, D), dtype=np.float32) / 32,
        "bv": np.zeros(D, np.float32),
        "Wo": rng.standard_normal((D, D), dtype=np.float32) / 32,
        "bo": np.zeros(D, np.float32),
    }
    got = kernel(**ins)
    ref = _numpy_fallback(**ins)
    err = np.abs(got - ref).max() / np.abs(ref).max()
    print("self-check relmax:", err)


# revision 18
# speedup vs baseline: 1.2324x; 1.2324x over previous
"""Trainium2 Bass kernel for nn_AttentionBlock (sparse_attention, no-softmax).

Computation (per batch b):
    qh = (q @ Wq^T) split into 16 heads of dk=64     [S, D] -> [H, S, DK]
    kh, vh likewise
    scores = (qh @ kh^T) / sqrt(DK)                  [H, S, S]
    p      = scores * A^T                            (elementwise structural mask)
    x      = p @ vh                                  [H, S, DK] -> [S, D]
    out    = x @ Wo^T + bo                           [S, D]

Sharding over 8 NeuronCores: data-parallel over batch (B=2) x tensor-parallel
over heads (16 heads -> 4 per core). Each core projects q/k/v for its 4 heads
(column-parallel), runs masked attention for them, and applies its 256-column
slice of the output projection (row-parallel), producing a full-shape partial
output. Host sums the 4 partials per batch.

Implementation notes:
- Activations are shipped pre-transposed ([D, S]) so every matmul contraction
  dim lands on SBUF partitions with no on-device transposes; 1/sqrt(DK) is
  folded into the mask A on the host.
- The whole data path runs in fp16 with fp32 PSUM accumulation (all operands
  here are O(1)-O(100), well inside fp16 range; measured end-to-end error is
  a few 1e-4). fp16 is the same PE stream rate as bf16/f32r but, being
  2-byte, additionally halves DMA/SBUF traffic and legalizes PE quadrant
  packing (tile_position), which f32/f32r reject.
- Heads are stored as pairs on the partition axis (head 2j on partitions
  0:63, head 2j+1 on 64:127). The K=64 score matmuls of a pair run
  concurrently in the upper/lower PE row-quadrants (tile_position (0,0) /
  (64,0)); the M=64 p@v matmuls of a pair run concurrently in left/right
  col-quadrants into one PSUM bank (tile_position (0,0) / (0,64)).
- The mask multiply is the throughput-critical elementwise stage; it is
  spread over three engines: DVE (straight out of PSUM), and a ScalarE
  PSUM->SBUF bounce feeding GPSIMD, alternating per key-block.
- Projection work for the next/previous query block is interleaved into the
  attention loop so no engine drains the pipeline at block boundaries.
"""

import numpy as np

import concourse.mybir as mybir
import concourse.tile as tile
from concourse import bacc, bass_utils

B, S, D, H = 2, 2048, 1024, 16
NCORES = 8
GROUPS = NCORES // B          # 4 head-groups
HPC = H // GROUPS             # 4 heads per core
DK = D // H                   # 64
HD = HPC * DK                 # 256 head-dim columns per core
NPAIR = HPC // 2              # 2 head pairs per core
SCALE = 1.0 / np.sqrt(DK)

P = 128                       # SBUF partitions
QB = 512                      # query block
NQB = S // QB                 # 4
KBLK = 128                    # key block
NKB = S // KBLK               # 16
NKT = D // P                  # 8 contraction chunks for projections
AGRP = 4                      # key-blocks per A-tile DMA / interleave group
NGRP = NKB // AGRP            # 4 groups

f32 = mybir.dt.float32
f16 = mybir.dt.float16

_CACHED = None  # built module, reused across kernel() calls
TRACE = False         # set True (e.g. from test.py) to profile the NEFF
LAST_RESULTS = None   # BassKernelResults of the most recent run


def _build():
    nc = bacc.Bacc("TRN2", target_bir_lowering=False)

    qT = nc.dram_tensor("qT", [D, S], f16, kind="ExternalInput")
    kT = nc.dram_tensor("kT", [D, S], f16, kind="ExternalInput")
    vT = nc.dram_tensor("vT", [D, S], f16, kind="ExternalInput")
    Asc = nc.dram_tensor("Asc", [S, S], f16, kind="ExternalInput")
    wq = nc.dram_tensor("wq", [D, HD], f16, kind="ExternalInput")
    wk = nc.dram_tensor("wk", [D, HD], f16, kind="ExternalInput")
    wv = nc.dram_tensor("wv", [D, HD], f16, kind="ExternalInput")
    wo = nc.dram_tensor("wo", [HD, D], f16, kind="ExternalInput")
    out = nc.dram_tensor("out", [S, D], f32, kind="ExternalOutput")

    qT_r = qT.rearrange("(kt p) s -> p kt s", p=P)
    kT_r = kT.rearrange("(kt p) s -> p kt s", p=P)
    vT_r = vT.rearrange("(kt p) s -> p kt s", p=P)
    wq_r = wq.rearrange("(kt p) c -> p kt c", p=P)
    wk_r = wk.rearrange("(kt p) c -> p kt c", p=P)
    wv_r = wv.rearrange("(kt p) c -> p kt c", p=P)
    wo_r = wo.rearrange("(ck p) e -> p ck e", p=P)
    A_r = Asc.rearrange("(kb p) q -> p kb q", p=P)

    with tile.TileContext(nc) as tc:
        with (
            tc.tile_pool(name="persist", bufs=1) as pp,
            tc.tile_pool(name="stream", bufs=2) as sp,
            tc.tile_pool(name="psU", bufs=6, space="PSUM") as psU,   # univ [128,512]
            tc.tile_pool(name="psX", bufs=1, space="PSUM") as psX,   # xT accumulators
        ):
            wk_sb = pp.tile([P, NKT, HD], f16, tag="wk")
            wv_sb = pp.tile([P, NKT, HD], f16, tag="wv")
            wq_sb = pp.tile([P, NKT, HD], f16, tag="wq")
            wo_sb = pp.tile([P, HD // P, D], f16, tag="wo")

            # head-PAIR layout: pair j holds head 2j on partitions 0:64 and
            # head 2j+1 on 64:128 — the layout quadrant packing requires
            khT_sb = pp.tile([P, NPAIR, S], f16, tag="khT")
            vh_sb = pp.tile([P, NKB, HD], f16, tag="vh")     # [ks%128, kb, c]

            def kv_proj(st, kT_pre=None, vT_pre=None):
                """Project k and v for s-slice st (512 rows of S)."""
                sl = slice(st * QB, (st + 1) * QB)
                kT_sb = kT_pre
                if kT_sb is None:
                    kT_sb = sp.tile([P, NKT, QB], f16, tag="xin", bufs=4, name="kT_sb")
                    nc.sync.dma_start(kT_sb[:], kT_r[:, :, sl])
                for ct in range(NPAIR):
                    pk = psU.tile([P, QB], f32, tag="u", name="pk")
                    for kt in range(NKT):
                        nc.tensor.matmul(
                            pk[:], wk_sb[:, kt, ct * P:(ct + 1) * P], kT_sb[:, kt, :],
                            start=(kt == 0), stop=(kt == NKT - 1),
                        )
                    # pair layout: both copies stay partition-aligned
                    nc.scalar.copy(khT_sb[0:DK, ct, sl], pk[0:DK, :])
                    nc.scalar.copy(khT_sb[DK:P, ct, sl], pk[DK:P, :])

                vT_sb = vT_pre
                if vT_sb is None:
                    vT_sb = sp.tile([P, NKT, QB], f16, tag="xin", bufs=4, name="vT_sb")
                    nc.sync.dma_start(vT_sb[:], vT_r[:, :, sl])
                for ssub in range(QB // P):
                    kb = st * (QB // P) + ssub
                    pv = psU.tile([P, HD], f32, tag="u", name="pv")
                    for kt in range(NKT):
                        nc.tensor.matmul(
                            pv[:], vT_sb[:, kt, ssub * P:(ssub + 1) * P], wv_sb[:, kt, :],
                            start=(kt == 0), stop=(kt == NKT - 1),
                        )
                    nc.scalar.copy(vh_sb[:, kb, :], pv[:])

            def q_proj(qb):
                """Project q for query block qb -> qhT pair tile [128, 2, 512]."""
                qsl = slice(qb * QB, (qb + 1) * QB)
                qT_sb = sp.tile([P, NKT, QB], f16, tag="xin", bufs=4, name="qT_sb")
                nc.sync.dma_start(qT_sb[:], qT_r[:, :, qsl])
                qhT_sb = sp.tile([P, NPAIR, QB], f16, tag="qh", name="qhT_sb")
                for ct in range(NPAIR):
                    pq = psU.tile([P, QB], f32, tag="u", name="pq")
                    for kt in range(NKT):
                        nc.tensor.matmul(
                            pq[:], wq_sb[:, kt, ct * P:(ct + 1) * P], qT_sb[:, kt, :],
                            start=(kt == 0), stop=(kt == NKT - 1),
                        )
                    nc.scalar.copy(qhT_sb[0:DK, ct, :], pq[0:DK, :])
                    nc.scalar.copy(qhT_sb[DK:P, ct, :], pq[DK:P, :])
                return qhT_sb

            def o_proj_chain(xts, qb, ssub):
                """One 128-row slice of the output projection for query block qb."""
                osb = sp.tile([P, D], f32, tag="osb", bufs=3, name="osb")
                for et in range(D // QB):
                    po = psU.tile([P, QB], f32, tag="u", name="po")
                    for ck in range(HD // P):
                        nc.tensor.matmul(
                            po[:],
                            xts[:, ck, ssub * P:(ssub + 1) * P],
                            wo_sb[:, ck, et * QB:(et + 1) * QB],
                            start=(ck == 0), stop=(ck == HD // P - 1),
                        )
                    nc.scalar.copy(osb[:, et * QB:(et + 1) * QB], po[:])
                nc.sync.dma_start(
                    out[qb * QB + ssub * P:qb * QB + (ssub + 1) * P, :], osb[:]
                )

            # ---- pipeline ---------------------------------------------------------
            # prologue DMAs in dependency-first order so the PE starts ASAP
            nc.sync.dma_start(wk_sb[:], wk_r[:])
            kT0 = sp.tile([P, NKT, QB], f16, tag="xin", bufs=4, name="kT_sb")
            nc.sync.dma_start(kT0[:, 0:NKT // 2, :], kT_r[:, 0:NKT // 2, 0:QB])
            nc.sync.dma_start(kT0[:, NKT // 2:, :], kT_r[:, NKT // 2:, 0:QB])
            nc.sync.dma_start(wv_sb[:], wv_r[:])
            vT0 = sp.tile([P, NKT, QB], f16, tag="xin", bufs=4, name="vT_sb")
            nc.sync.dma_start(vT0[:], vT_r[:, :, 0:QB])
            nc.sync.dma_start(wq_sb[:], wq_r[:])
            A0 = sp.tile([P, AGRP, QB], f16, tag="A", bufs=3, name="A_sb")
            nc.sync.dma_start(A0[:], A_r[:, 0:AGRP, 0:QB])

            kv_proj(0, kT_pre=kT0, vT_pre=vT0)
            qhT_cur = q_proj(0)
            nc.sync.dma_start(wo_sb[:], wo_r[:])

            pend_xts = None    # (xts tile, qb) awaiting output projection
            qhT_next = None

            for qb in range(NQB):
                qsl = slice(qb * QB, (qb + 1) * QB)
                xt = psX.tile([P, NPAIR, QB], f32, tag="xt", name="xt")  # 2 banks
                xts = sp.tile([P, NPAIR, QB], f16, tag="xts", bufs=2, name="xts")
                for kbg in range(NGRP):
                    if qb == 0 and kbg == 0:
                        A_sb = A0
                    else:
                        A_sb = sp.tile([P, AGRP, QB], f16, tag="A", bufs=3, name="A_sb")
                        nc.sync.dma_start(
                            A_sb[:], A_r[:, kbg * AGRP:(kbg + 1) * AGRP, qsl]
                        )
                    for i in range(AGRP):
                        kb = kbg * AGRP + i
                        ksl = slice(kb * KBLK, (kb + 1) * KBLK)
                        # scores: both heads of a pair run concurrently in the
                        # upper/lower PE row-quadrants
                        scs = []
                        for j in range(NPAIR):
                            sc_e = psU.tile([P, QB], f32, tag="u", name="sc_e")
                            nc.tensor.matmul(
                                sc_e[:], khT_sb[0:DK, j, ksl], qhT_cur[0:DK, j, :],
                                start=True, stop=True, tile_position=(0, 0),
                            )
                            sc_o = psU.tile([P, QB], f32, tag="u", name="sc_o")
                            nc.tensor.matmul(
                                sc_o[:], khT_sb[DK:P, j, ksl], qhT_cur[DK:P, j, :],
                                start=True, stop=True, tile_position=(DK, 0),
                            )
                            scs += [sc_e, sc_o]
                        # mask multiply, spread over DVE / (ScalarE+GPSIMD):
                        # heads 0,1 on DVE; heads 2,3 alternate by key-block
                        pts = []
                        for h in range(HPC):
                            pt = sp.tile([P, QB], f16, tag="pt", bufs=8, name="pt")
                            use_gp = h == 3 or (h == 2 and kb % 2 == 0)
                            if use_gp:
                                sc_sb = sp.tile([P, QB], f32, tag="scb", bufs=4,
                                                name="sc_sb")
                                nc.scalar.copy(sc_sb[:], scs[h][:])
                                nc.gpsimd.tensor_tensor(
                                    pt[:], sc_sb[:], A_sb[:, i, :],
                                    mybir.AluOpType.mult,
                                )
                            else:
                                nc.vector.tensor_tensor(
                                    pt[:], scs[h][:], A_sb[:, i, :],
                                    mybir.AluOpType.mult,
                                )
                            pts.append(pt)
                        # p @ v: both heads of a pair run concurrently in the
                        # left/right PE col-quadrants into one PSUM bank
                        for j in range(NPAIR):
                            nc.tensor.matmul(
                                xt[0:DK, j, :],
                                vh_sb[:, kb, (2 * j) * DK:(2 * j + 1) * DK],
                                pts[2 * j],
                                start=(kb == 0), stop=(kb == NKB - 1),
                                tile_position=(0, 0),
                            )
                            nc.tensor.matmul(
                                xt[DK:P, j, :],
                                vh_sb[:, kb, (2 * j + 1) * DK:(2 * j + 2) * DK],
                                pts[2 * j + 1],
                                start=(kb == 0), stop=(kb == NKB - 1),
                                tile_position=(0, DK),
                            )
                            if kb == NKB - 1:
                                # drain this pair's accumulator immediately
                                nc.scalar.copy(xts[:, j, :], xt[:, j, :])

                    # interleave independent work into the elementwise-bound loop
                    if qb == 0:
                        if kbg < NGRP - 1:
                            kv_proj(kbg + 1)
                        else:
                            qhT_next = q_proj(1)
                    else:
                        if kbg < 2 and pend_xts is not None:
                            xts_p, qb_p = pend_xts
                            o_proj_chain(xts_p, qb_p, 2 * kbg)
                            o_proj_chain(xts_p, qb_p, 2 * kbg + 1)
                            if kbg == 1:
                                pend_xts = None
                        elif kbg == NGRP - 1 and qb < NQB - 1:
                            qhT_next = q_proj(qb + 1)

                pend_xts = (xts, qb)
                qhT_cur, qhT_next = qhT_next, None

            # drain the last query block's output projection
            xts_p, qb_p = pend_xts
            for ssub in range(QB // P):
                o_proj_chain(xts_p, qb_p, ssub)

    nc.compile()
    return nc


def _numpy_fallback(q, k, v, A, Wq, bq, Wk, bk, Wv, bv, Wo, bo):
    def proj(x, W, b):
        y = x @ W.T + b
        return y.reshape(B, S, H, DK).transpose(0, 2, 1, 3)

    qh, kh, vh = proj(q, Wq, bq), proj(k, Wk, bk), proj(v, Wv, bv)
    scores = np.einsum("bhqd,bhkd->bhqk", qh, kh) * np.float32(SCALE)
    p = scores * A.T
    x = np.einsum("bhqk,bhkd->bhqd", p, vh)
    x = x.transpose(0, 2, 1, 3).reshape(B, S, D)
    return (x @ Wo.T + bo).astype(np.float32)


def kernel(**inputs):
    q = np.asarray(inputs["q"], dtype=np.float32)
    k = np.asarray(inputs["k"], dtype=np.float32)
    v = np.asarray(inputs["v"], dtype=np.float32)
    A = np.asarray(inputs["A"], dtype=np.float32)
    Wq = np.asarray(inputs["Wq"], dtype=np.float32)
    Wk = np.asarray(inputs["Wk"], dtype=np.float32)
    Wv = np.asarray(inputs["Wv"], dtype=np.float32)
    Wo = np.asarray(inputs["Wo"], dtype=np.float32)
    bq, bk, bv, bo = (np.asarray(inputs[n], dtype=np.float32) for n in ("bq", "bk", "bv", "bo"))

    # The device kernel folds zero biases away (spec fills them with zeros);
    # fall back to a host reference in the (unused) nonzero-bias case.
    if any(np.any(b) for b in (bq, bk, bv)):
        return _numpy_fallback(q, k, v, A, Wq, bq, Wk, bk, Wv, bv, Wo, bo)

    global _CACHED
    if _CACHED is None:
        _CACHED = _build()
    nc = _CACHED

    Asc = np.ascontiguousarray((A * np.float32(SCALE)).astype(np.float16))
    in_maps = []
    for c in range(NCORES):
        b, g = divmod(c, GROUPS)
        hsl = slice(g * HD, (g + 1) * HD)
        in_maps.append({
            "qT": np.ascontiguousarray(q[b].T.astype(np.float16)),
            "kT": np.ascontiguousarray(k[b].T.astype(np.float16)),
            "vT": np.ascontiguousarray(v[b].T.astype(np.float16)),
            "Asc": Asc,
            "wq": np.ascontiguousarray(Wq[hsl].T.astype(np.float16)),
            "wk": np.ascontiguousarray(Wk[hsl].T.astype(np.float16)),
            "wv": np.ascontiguousarray(Wv[hsl].T.astype(np.float16)),
            "wo": np.ascontiguousarray(Wo[:, hsl].T.astype(np.float16)),
        })

    res = bass_utils.run_bass_kernel_spmd(
        nc, in_maps, core_ids=list(range(NCORES)), trace=TRACE
    )
    global LAST_RESULTS
    LAST_RESULTS = res

    out = np.zeros((B, S, D), dtype=np.float32)
    for c in range(NCORES):
        out[c // GROUPS] += res.results[c]["out"]
    out += bo
    return out


if __name__ == "__main__":
    rng = np.random.default_rng(0)
    ins = {
        "q": rng.standard_normal((B, S, D), dtype=np.float32),
        "k": rng.standard_normal((B, S, D), dtype=np.float32),
        "v": rng.standard_normal((B, S, D), dtype=np.float32),
        "A": rng.random((S, S), dtype=np.float32),
        "Wq": rng.standard_normal((D, D), dtype=np.float32) / 32,
        "bq": np.zeros(D, np.float32),
        "Wk": rng.standard_normal((D, D), dtype=np.float32) / 32,
        "bk": np.zeros(D, np.float32),
        "Wv": rng.standard_normal((D, D), dtype=np.float32) / 32,
        "bv": np.zeros(D, np.float32),
        "Wo": rng.standard_normal((D, D), dtype=np.float32) / 32,
        "bo": np.zeros(D, np.float32),
    }
    got = kernel(**ins)
    ref = _numpy_fallback(**ins)
    err = np.abs(got - ref).max() / np.abs(ref).max()
    print("self-check relmax:", err)
